# revision 7
# baseline (speedup 1.0000x reference)
"""Trainium2 Bass kernel for CrossAttention + residual + LayerNorm.

Problem: B=4, Sq=Skv=2048, D=512, H=8 heads (dh=64), fp32 I/O.

Sharding (8 cores, no collectives): core c handles batch b=c//2 and query-row
half r=c%2 (1024 q rows). Each core loads its x slice [1024,512], the full
cross-attn input for its batch [2048,512], and all weights; computes
q/k/v projections, per-head softmax(q k^T / 8) @ v, output projection,
residual add and layernorm for its 1024 rows. Host reassembles.

Layout strategy (all matmul operands bf16, fp32 PSUM accumulation):
  - x^T, ca^T built on-chip via PE transposes (cast to bf16 on PSUM evict)
  - q^T [d_out, q], k^T [d_out, kv] produced directly by the projections
  - scores computed TRANSPOSED: scores^T[kv, q] = k_h @ q_h^T, so softmax's
    kv-reduction becomes a matmul contraction (no cross-partition reduces)
  - head pairs (base partition 0 / 64, K=64) auto row-tile the PE array
  - exp via ScalarE on [128, 2048] PSUM tiles (one per kv-tile, both heads)
  - AV uses v as lhsT augmented with a ones column: out^T[65, q] where row 64
    is the softmax denominator; normalize via partition_broadcast + DVE mul,
    writing attn^T directly in the layout the output projection consumes.
"""
import sys

sys.path.insert(0, "/opt/trn_rl_repo")

from contextlib import ExitStack

import numpy as np

import concourse.bass as bass
import concourse.mybir as mybir
import concourse.tile as tile
from concourse import bacc
from concourse.bass_utils import run_bass_kernel_spmd
from concourse.masks import make_identity

B, SQ, SKV, D, H = 4, 2048, 2048, 512, 8
DH = D // H            # 64 head dim
P = 128
NCORES = 8
SQC = B * SQ // NCORES  # 1024 query rows per core
QT = SQC // P           # 8 q row tiles
KT = SKV // P           # 16 kv tiles
DT = D // P             # 4 embed tiles
HP = H // 2             # 4 head pairs
F32 = mybir.dt.float32
BF16 = mybir.dt.bfloat16
AF = mybir.ActivationFunctionType
EPS = 1e-5
SCALE = DH ** -0.5      # 0.125


def _emit(tc, ctx, io, dbg=None):
    nc = tc.nc
    x_d, ca_d, wq_d, bq_d, wkv_d, bkv_d, wo_d, bo_d, gm_d, bt_d, out_d = io

    const = ctx.enter_context(tc.tile_pool(name="const", bufs=1))
    psA = ctx.enter_context(tc.tile_pool(name="psA", bufs=1, space="PSUM"))
    psB = ctx.enter_context(tc.tile_pool(name="psB", bufs=4, space="PSUM"))

    # ---------- constants / persistent tensors ----------
    ident_f = const.tile([P, P], F32)
    make_identity(nc, ident_f)
    ident_b = const.tile([P, P], BF16)
    nc.vector.tensor_copy(ident_b, ident_f)

    x_sb = const.tile([P, QT, D], F32)          # residual + transpose source
    nc.sync.dma_start(out=x_sb, in_=x_d.rearrange("(t p) d -> p t d", p=P))

    wq_sb = const.tile([P, DT, D], BF16)
    nc.gpsimd.dma_start(out=wq_sb, in_=wq_d.rearrange("(t p) n -> p t n", p=P))
    wkv_sb = const.tile([P, DT, 2 * D], BF16)
    nc.gpsimd.dma_start(out=wkv_sb, in_=wkv_d.rearrange("(t p) n -> p t n", p=P))
    wo_sb = const.tile([P, DT, D], BF16)
    nc.gpsimd.dma_start(out=wo_sb, in_=wo_d.rearrange("(t p) n -> p t n", p=P))

    bq_sb = const.tile([P, DT], F32)
    nc.sync.dma_start(out=bq_sb, in_=bq_d.rearrange("(t p) -> p t", p=P))
    bk_sb = const.tile([P, DT], F32)
    nc.sync.dma_start(out=bk_sb, in_=bkv_d[0:D].rearrange("(t p) -> p t", p=P))

    def bcast(src_ap, tag):  # replicate a [D] vector over all 128 partitions
        t = const.tile([P, D], F32, tag=tag)
        rep = bass.AP(tensor=src_ap.tensor, offset=src_ap.offset,
                      ap=[[0, P]] + list(src_ap.ap))
        nc.gpsimd.dma_start(out=t, in_=rep)
        return t

    bv_bc = bcast(bkv_d[D:2 * D], "bv_bc")
    bo_bc = bcast(bo_d[:], "bo_bc")
    gm_bc = bcast(gm_d[:], "gm_bc")
    bt_bc = bcast(bt_d[:], "bt_bc")

    eps_t = const.tile([P, 1], F32)
    nc.vector.memset(eps_t, EPS)

    p1 = tc.tile_pool(name="p1", bufs=1)
    p1ctx = p1.__enter__()
    ca_scope = tc.tile_pool(name="ca_pool", bufs=3)
    ca_pool = ca_scope.__enter__()
    xT = p1ctx.tile([P, DT, SQC], BF16)
    caT = p1ctx.tile([P, DT, SKV], BF16)
    qT = const.tile([P, DT, SQC], BF16)
    kT = const.tile([P, DT, SKV], BF16)
    v_aug = const.tile([P, KT, H, DH + 1], BF16)
    attnT = const.tile([P, DT, SQC], BF16)

    nc.vector.memset(v_aug[:, :, :, DH:DH + 1], 1.0)  # ones column per head

    # ---------- phase 1: transposes + projections ----------
    # x^T (fp32 in, bf16 out)
    for rt in range(QT):
        pt = psB.tile([P, DT, P], F32, tag="ps1")
        for c in range(DT):
            nc.tensor.transpose(pt[:, c, :], x_sb[:, rt, c * P:(c + 1) * P], ident_f)
        nc.vector.tensor_copy(xT[:, :, rt * P:(rt + 1) * P], pt)

    # ca^T (cast-DMA to bf16, then PE transpose)
    for t in range(KT):
        ca_t = ca_pool.tile([P, D], BF16)
        nc.gpsimd.dma_start(out=ca_t, in_=ca_d[t * P:(t + 1) * P, :])
        pt = psB.tile([P, DT, P], BF16, tag="ps1")
        for c in range(DT):
            nc.tensor.transpose(pt[:, c, :], ca_t[:, c * P:(c + 1) * P], ident_b)
        nc.vector.tensor_copy(caT[:, :, t * P:(t + 1) * P], pt)

    # q^T[m*128:(m+1)*128, :] = (x @ Wq + bq)^T : lhsT=Wq[:,m], rhs=x^T
    for m in range(DT):
        for qc in range(SQC // 512):
            acc = psB.tile([P, 512], F32, tag="ps1")
            for kd in range(DT):
                nc.tensor.matmul(acc, wq_sb[:, kd, m * P:(m + 1) * P],
                                 xT[:, kd, qc * 512:(qc + 1) * 512],
                                 start=(kd == 0), stop=(kd == DT - 1))
            nc.scalar.activation(qT[:, m, qc * 512:(qc + 1) * 512], acc,
                                 AF.Identity, bias=bq_sb[:, m:m + 1])

    # k^T
    for m in range(DT):
        for cc in range(SKV // 512):
            acc = psB.tile([P, 512], F32, tag="ps1")
            for kd in range(DT):
                nc.tensor.matmul(acc, wkv_sb[:, kd, m * P:(m + 1) * P],
                                 caT[:, kd, cc * 512:(cc + 1) * 512],
                                 start=(kd == 0), stop=(kd == DT - 1))
            nc.scalar.activation(kT[:, m, cc * 512:(cc + 1) * 512], acc,
                                 AF.Identity, bias=bk_sb[:, m:m + 1])

    # v[t] = ca @ Wv + bv  -> v_aug[:, t, h, 0:64]
    for t in range(KT):
        acc = psB.tile([P, 512], F32, tag="ps1")
        for kd in range(DT):
            nc.tensor.matmul(acc, caT[:, kd, t * P:(t + 1) * P],
                             wkv_sb[:, kd, D:2 * D],
                             start=(kd == 0), stop=(kd == DT - 1))
        nc.vector.tensor_add(
            v_aug[:, t, :, 0:DH],
            acc.rearrange("p (h d) -> p h d", h=H),
            bv_bc.rearrange("p (h d) -> p h d", h=H))

    ca_scope.__exit__(None, None, None)
    p1.__exit__(None, None, None)

    # ---------- phase 2: attention, one head-pair at a time ----------
    probs_pool = ctx.enter_context(tc.tile_pool(name="probs", bufs=1))
    work = ctx.enter_context(tc.tile_pool(name="work", bufs=2))
    ep = ctx.enter_context(tc.tile_pool(name="ep", bufs=2))
    for hp in range(HP):
        h0, h1 = 2 * hp, 2 * hp + 1
        probs = probs_pool.tile([P, KT, 2 * SQC], BF16)
        # scores^T and exp: psum cols [h_local*1024 + qc*512]
        for t in range(KT):
            ss = psA.tile([P, 2 * SQC], F32)
            for qc in range(SQC // 512):
                nc.tensor.matmul(
                    ss[:, qc * 512:(qc + 1) * 512],
                    kT[0:DH, hp, t * P:(t + 1) * P],
                    qT[0:DH, hp, qc * 512:(qc + 1) * 512])
                nc.tensor.matmul(
                    ss[:, SQC + qc * 512:SQC + (qc + 1) * 512],
                    kT[DH:P, hp, t * P:(t + 1) * P],
                    qT[DH:P, hp, qc * 512:(qc + 1) * 512])
            nc.scalar.activation(probs[:, t, :], ss, AF.Exp, scale=SCALE)
        if dbg is not None and hp == 0:
            nc.gpsimd.dma_start(out=dbg["probs"], in_=probs)

        # AV: out^T[65, 512] per (head, q-chunk); row 64 = denominator
        for hl, h in ((0, h0), (1, h1)):
            for qc in range(SQC // 512):
                av = psB.tile([P, 512], F32, tag="ps1")
                for t in range(KT):
                    nc.tensor.matmul(
                        av[0:DH + 1, :], v_aug[:, t, h, :],
                        probs[:, t, hl * SQC + qc * 512:hl * SQC + (qc + 1) * 512],
                        start=(t == 0), stop=(t == KT - 1))
                den = work.tile([DH + 1, 512], F32, tag="den")
                dbc = work.tile([DH, 512], F32, tag="dbc")
                nc.vector.reciprocal(den[DH:DH + 1, :], av[DH:DH + 1, :])
                # partition_broadcast only reads partition 0 on HW: hop 64->0
                nc.gpsimd.tensor_copy(den[0:1, :], den[DH:DH + 1, :])
                nc.gpsimd.partition_broadcast(dbc, den[0:1, :])
                nc.vector.tensor_mul(
                    attnT[hl * DH:(hl + 1) * DH, hp, qc * 512:(qc + 1) * 512],
                    av[0:DH, :], dbc)

    # ---------- phase 3: output projection + residual + layernorm ----------
    for qt in range(QT):
        acc = psB.tile([P, 512], F32, tag="ps1")
        for kd in range(DT):
            nc.tensor.matmul(acc, attnT[:, kd, qt * P:(qt + 1) * P],
                             wo_sb[:, kd], start=(kd == 0), stop=(kd == DT - 1))
        t1 = ep.tile([P, D], F32, tag="t1")
        nc.vector.tensor_add(t1, acc, x_sb[:, qt, :])
        nc.vector.tensor_add(t1, t1, bo_bc)
        stats = ep.tile([P, 6], F32, tag="stats")
        nc.vector.bn_stats(stats, t1)
        mv = ep.tile([P, 2], F32, tag="mv")
        nc.vector.bn_aggr(mv, stats)
        rstd = ep.tile([P, 1], F32, tag="rstd")
        nc.scalar.activation(rstd, mv[:, 1:2], AF.Sqrt, bias=eps_t)
        nc.vector.reciprocal(rstd, rstd)
        norm = ep.tile([P, D], F32, tag="norm")
        nc.vector.tensor_scalar(norm, t1, scalar1=mv[:, 0:1], scalar2=rstd,
                                op0=mybir.AluOpType.subtract,
                                op1=mybir.AluOpType.mult)
        nc.vector.tensor_mul(norm, norm, gm_bc)
        nc.vector.tensor_add(norm, norm, bt_bc)
        nc.sync.dma_start(out_d[qt * P:(qt + 1) * P, :], norm)

    if dbg is not None:
        nc.gpsimd.dma_start(out=dbg["qT"], in_=qT)
        nc.gpsimd.dma_start(out=dbg["kT"], in_=kT)
        nc.gpsimd.dma_start(out=dbg["v_aug"], in_=v_aug)
        nc.gpsimd.dma_start(out=dbg["attnT"], in_=attnT)


def _build(debug=False):
    nc = bacc.Bacc("TRN2", target_bir_lowering=False, debug=False,
                   num_devices=NCORES)
    io = (
        nc.dram_tensor("x", [SQC, D], F32, kind="ExternalInput").ap(),
        nc.dram_tensor("ca", [SKV, D], F32, kind="ExternalInput").ap(),
        nc.dram_tensor("wq", [D, D], F32, kind="ExternalInput").ap(),
        nc.dram_tensor("bq", [D], F32, kind="ExternalInput").ap(),
        nc.dram_tensor("wkv", [D, 2 * D], F32, kind="ExternalInput").ap(),
        nc.dram_tensor("bkv", [2 * D], F32, kind="ExternalInput").ap(),
        nc.dram_tensor("wo", [D, D], F32, kind="ExternalInput").ap(),
        nc.dram_tensor("bo", [D], F32, kind="ExternalInput").ap(),
        nc.dram_tensor("gamma", [D], F32, kind="ExternalInput").ap(),
        nc.dram_tensor("beta", [D], F32, kind="ExternalInput").ap(),
        nc.dram_tensor("out", [SQC, D], F32, kind="ExternalOutput").ap(),
    )
    dbg = None
    if debug:
        dbg = {
            "qT": nc.dram_tensor("d_qT", [P, DT, SQC], F32, kind="ExternalOutput").ap(),
            "kT": nc.dram_tensor("d_kT", [P, DT, SKV], F32, kind="ExternalOutput").ap(),
            "v_aug": nc.dram_tensor("d_v", [P, KT, H, DH + 1], F32, kind="ExternalOutput").ap(),
            "attnT": nc.dram_tensor("d_aT", [P, DT, SQC], F32, kind="ExternalOutput").ap(),
            "probs": nc.dram_tensor("d_pr", [P, KT, 2 * SQC], F32, kind="ExternalOutput").ap(),
        }
    with tile.TileContext(nc) as tc, ExitStack() as ctx:
        _emit(tc, ctx, io, dbg)
    nc.compile()
    return nc


_CACHE = {}


def _get_nc():
    if "nc" not in _CACHE:
        _CACHE["nc"] = _build()
    return _CACHE["nc"]


def kernel(layer_input, cross_attn_input, Wq, bq, Wkv, bkv, Wo, bo, gamma,
           beta, trace=False):
    f32 = np.float32
    layer_input = np.ascontiguousarray(layer_input, dtype=f32)
    cross_attn_input = np.ascontiguousarray(cross_attn_input, dtype=f32)
    shared = {
        "wq": np.ascontiguousarray(Wq, f32),
        "bq": np.ascontiguousarray(bq, f32),
        "wkv": np.ascontiguousarray(Wkv, f32),
        "bkv": np.ascontiguousarray(bkv, f32),
        "wo": np.ascontiguousarray(Wo, f32),
        "bo": np.ascontiguousarray(bo, f32),
        "gamma": np.ascontiguousarray(gamma, f32),
        "beta": np.ascontiguousarray(beta, f32),
    }
    in_maps = []
    for c in range(NCORES):
        b, r = c // 2, c % 2
        in_maps.append({
            "x": np.ascontiguousarray(layer_input[b, r * SQC:(r + 1) * SQC, :]),
            "ca": np.ascontiguousarray(cross_attn_input[b]),
            **shared,
        })
    nc = _get_nc()
    res = run_bass_kernel_spmd(nc, in_maps, core_ids=list(range(NCORES)),
                               trace=trace)
    out = np.empty((B, SQ, D), np.float32)
    for c in range(NCORES):
        b, r = c // 2, c % 2
        out[b, r * SQC:(r + 1) * SQC, :] = res.results[c]["out"]
    if trace:
        return out, res
    return out
